# revision 44
# baseline (speedup 1.0000x reference)
"""Multi-head causal self-attention on 8 Trainium2 NeuronCores (Bass/Tile).

Problem: y = proj(softmax(causal_mask(Q K^T / sqrt(D))) V) for B=2, T=2048,
C=1024, H=16 heads, D=64.

Sharding (tensor-parallel over heads, 8-way):
  - Core i owns heads {2i, 2i+1}: computes qT/kT/vT for its heads over both
    batches (full x, its 128-column slice of Wqkv) and runs causal attention
    per head fully on-core, producing *unnormalized* yT_local plus the
    reciprocal softmax denominators (head-dims on partitions, time free).
  - Two 8-way AllToAlls reshard head-split -> time-split. Each payload row
    group is [65, 512]: 64 rows of unnormalized y^T plus one row of
    reciprocal denominators. Core j ends with yT_full [128, 4096] (all 16
    heads) for time-slice j, normalizes it there (K=1 ones-matmul broadcast
    of the recip rows + one vector multiply per source core), and computes
    its [512, 1024] slice of y @ Wproj.
  - The host concatenates the 8 time-slices into [2, 2048, 1024].

All matmul operands are bf16 (PE streams 1 col/cycle vs 2-4 for
fp32/fp32r; measured on HW: fp32r 128x128 matmuls run at the full-fp32
4-cycle rate). Accumulation stays fp32 in PSUM. bf16 also halves HBM and
collective traffic. Error: 5.9e-3 measured vs 2e-2 tolerance.

Post-collective normalization: each (head, batch) ships unnormalized y^T
plus per-query reciprocal denominators (row 64 of the payload, per-j
reciprocals so the last j's chain into the collective trigger is short).
The receiver broadcasts recip rows across the 64 partitions of each head
half with one K=1 ones-matmul per source core and multiplies in place;
the h=0 half normalizes during the second collective's transfer window.

Scheduling notes (the tile scheduler reorders within engine queues by
priority, so program order alone does not pin queue order):
  - tc.tile_wait_until pins the finale (ytf/rsb pulls, normalize, proj)
    behind the attention phases; without it the scheduler hoists
    collective-gated DMAs/matmuls into the in-order PE/sync streams and
    stalls them ~50us on the first AllToAll.
  - The h=0 pulls ride the scalar queue (idle after its last exp); the
    sync queue is still draining attn(1,1) stage writes then.
  - Mesh AllToAlls make little progress while compute runs (~12GB/s
    overlapped vs ~22GB/s quiet), so A2A#1 hides under attn(1,*) and
    only A2A#2 (~25-35us) is exposed. AllGather with a Shared-HBM output
    was tried and is catastrophically slow (~2.4s); power/gpio throttle
    caps the PE at ~50-65% duty for much of the run, which is why matmul
    streams average ~440ns/512 cols instead of the unthrottled 237ns.

Attention is computed transposed (S^T[k, q], keys on partitions): no
transposes in the attention path, exp on ScalarE straight out of PSUM,
softmax denominators come free from a ones column appended to V (row 64
of the P@V accumulator). Causality is exact: S^T blocks strictly above
the diagonal are skipped; diagonal blocks use a restricted column range
plus a triangular multiplicative mask after exp. Blocks are processed in
1024-wide pairs (full and diagonal alike) so one ACTIVATE covers two
blocks; gaps between paired diagonal blocks exp garbage that no PV
matmul ever reads. Chunks are software-pipelined one deep (S/exp of
chunk i+1 issue before PV of chunk i) so ScalarE latency hides behind
the in-order PE queue.

DMA discipline: bulk loads (x, Wqkv, Wproj, ytf pulls, out) are single
coalesced multi-tile transfers on the sync queue; attention-tail DMAs
(a2a stage writes, recip rows) also ride the sync queue in data-ready
order; denominator gathers sit on the (otherwise idle) GpSimd queue so
the collective triggers never wait behind descriptor pushes.
"""

import numpy as np
import ml_dtypes

import concourse.bass as bass
import concourse.mybir as mybir
import concourse.tile as tile
from concourse import bacc
from concourse import bass_utils

F32 = mybir.dt.float32
BF16 = mybir.dt.bfloat16
AF = mybir.ActivationFunctionType

B, T, C = 2, 2048, 1024
H, D = 16, 64
N_CORES = 8
HL = H // N_CORES        # heads per core = 2
NCT = C // 128           # contraction tiles = 8
NQ = T // 512            # q tiles per batch = 4
NK = T // 128            # k tiles per batch = 16
SCALE = 1.0 / float(np.sqrt(D))  # 0.125

_BUILD_CACHE = {}


def _drain(*gens):
    """Round-robin the generators until all are exhausted."""
    active = list(gens)
    while active:
        nxt = []
        for g in active:
            try:
                next(g)
                nxt.append(g)
            except StopIteration:
                pass
        active = nxt


def build_kernel(apply_pad_mask: bool):
    nc = bacc.Bacc(
        "TRN2", target_bir_lowering=False, debug=False, num_devices=N_CORES
    )
    xT = nc.dram_tensor("xT", [C, B * T], BF16, kind="ExternalInput").ap()
    wqkv = nc.dram_tensor("wqkv", [C, 3 * HL * D], BF16, kind="ExternalInput").ap()
    wo = nc.dram_tensor("wo", [C, C], BF16, kind="ExternalInput").ap()
    tri = nc.dram_tensor("tri", [128, 128], F32, kind="ExternalInput").ap()
    ident = nc.dram_tensor("ident", [128, 128], BF16, kind="ExternalInput").ap()
    padk = nc.dram_tensor("padk", [128, B * NK], F32, kind="ExternalInput").ap()
    out = nc.dram_tensor("out", [512, C], BF16, kind="ExternalOutput").ap()

    with tile.TileContext(nc) as tc:
        with (
            tc.tile_pool(name="const", bufs=1) as constp,
            tc.tile_pool(name="qk", bufs=1) as qkp,
            tc.tile_pool(name="vv", bufs=1) as vvp,
            tc.tile_pool(name="xw", bufs=1) as xwp,
            tc.tile_pool(name="work", bufs=2) as wk,
            tc.tile_pool(name="ps_ss", bufs=2, space="PSUM") as ps_ss,
            tc.tile_pool(name="ps_main", bufs=2, space="PSUM") as ps_main,
            tc.tile_pool(name="ps_y", bufs=2, space="PSUM") as ps_y,
            tc.tile_pool(name="dram", bufs=1, space="DRAM") as dram,
        ):
            # ---------------- constants ----------------
            tri_sb = constp.tile([128, 128], F32, name="tri_sb")
            id_sb = constp.tile([128, 128], BF16, name="id_sb")
            # head-half selectors for the recip broadcast matmuls, present
            # at partition bases 0 and 64 (PE stationary/moving bases must
            # match and be 0/32/64)
            ones_h0 = constp.tile([65, 128], BF16, name="ones_h0")
            ones_h1 = constp.tile([65, 128], BF16, name="ones_h1")
            for r in (0, 64):
                nc.vector.memset(ones_h0[r:r + 1, 0:64], 1.0)
                nc.vector.memset(ones_h0[r:r + 1, 64:128], 0.0)
                nc.vector.memset(ones_h1[r:r + 1, 0:64], 0.0)
                nc.vector.memset(ones_h1[r:r + 1, 64:128], 1.0)
            onesc_f = constp.tile([128, HL], F32, name="onesc_f")
            nc.vector.memset(onesc_f[:], 1.0)
            onesc = constp.tile([128, HL], BF16, name="onesc")
            nc.gpsimd.tensor_copy(onesc[:], onesc_f[:])
            if apply_pad_mask:
                padk_sb = constp.tile([128, B * NK], F32, name="padk_sb")
                nc.sync.dma_start(padk_sb[:], padk[:])

            a2a_in = [dram.tile([N_CORES, 65, 512], BF16, name=f"a2a_in{h}")
                      for h in range(HL)]
            a2a_out = [dram.tile([N_CORES, 65, 512], BF16, name=f"a2a_out{h}")
                       for h in range(HL)]

            # big coalesced input tiles: slice ct lives at cols [ct*W,(ct+1)*W)
            # wqkv loads per projection (v first) on the idle scalar queue:
            # the first matmul group only waits for the 0.25MB v-slice while
            # x streams in parallel on the sync queue
            wqkv_sb = xwp.tile([128, NCT * 3 * HL * D], BF16, name="wqkv_sb")
            for which in (2, 1, 0):
                # v-slice gates the program's first matmul: head of the sync
                # queue (earliest preamble); k/q ride the scalar queue
                eng = nc.sync if which == 2 else nc.scalar
                eng.dma_start(
                    wqkv_sb[:].rearrange("p (c k) -> p c k", c=NCT)[
                        :, :, which * 128:(which + 1) * 128],
                    wqkv[:].rearrange("(c p) k -> p c k", c=NCT)[
                        :, :, which * 128:(which + 1) * 128],
                )
            xt_sb = xwp.tile([128, NCT * B * T], BF16, name="xt_sb")

            def xw(ct):
                return wqkv_sb[:, ct * 3 * HL * D:(ct + 1) * 3 * HL * D]

            def xx(ct, b):
                return xt_sb[:, ct * B * T + b * T: ct * B * T + (b + 1) * T]

            qT = [None] * B
            kT = [None] * B
            V = [[None] * NK for _ in range(B)]

            def xload(b, c0, c1):
                nc.sync.dma_start(
                    xt_sb[:].rearrange("p (c t) -> p c t", c=NCT)[:, :, b * T + c0:b * T + c1],
                    xT[:].rearrange("(c p) t -> p c t", c=NCT)[:, :, b * T + c0:b * T + c1],
                )

            def qkv_emit(b):
                """Projections for batch b: yields between schedulable
                chunks so the PE stream can interleave with attention."""
                if b == 0:
                    xload(0, 0, 512)
                    # constants ride behind the critical first loads
                    nc.sync.dma_start(tri_sb[:], tri[:])
                    nc.sync.dma_start(id_sb[:], ident[:])
                    xload(0, 512, 1024)
                    xload(0, 1024, T)
                qT[b] = qkp.tile([128, T], BF16, name="qT", tag=f"qT{b}")
                kT[b] = qkp.tile([128, T], BF16, name="kT", tag=f"kT{b}")
                vT = qkp.tile([128, T], BF16, name="vT", tag="vT")
                dsts = {2: vT, 1: kT[b], 0: qT[b]}
                if b == 0:
                    # x streams in while qkv(0) runs: early groups only wait
                    # on 512 x-columns each, and low-column k/q groups fill
                    # the stalls before the later x chunks land
                    sched = [(2, (0,)), (2, (1,)), (1, (0, 1)), (2, (2, 3)),
                             (0, (0, 1)), (1, (2, 3)), (0, (2, 3))]
                else:
                    sched = [(2, (0, 1)), (2, (2, 3)), (1, (0, 1)),
                             (1, (2, 3)), (0, (0, 1)), (0, (2, 3))]
                for which, ns in sched:
                    dst = dsts[which]
                    if True:
                        p2 = [ps_main.tile([128, 512], F32, name="p_mm",
                                           tag="ps") for _ in ns]
                        for ct in range(NCT):
                            for ni, n in enumerate(ns):
                                nc.tensor.matmul(
                                    p2[ni][:],
                                    xw(ct)[:, which * 128:(which + 1) * 128],
                                    xx(ct, b)[:, n * 512:(n + 1) * 512],
                                    start=(ct == 0),
                                    stop=(ct == NCT - 1),
                                )
                        for ni, n in enumerate(ns):
                            nc.vector.tensor_copy(
                                dst[:, n * 512:(n + 1) * 512], p2[ni][:])
                            yield
                        if b == 0 and which == 2 and ns == (1,):
                            # batch-1 x arrives while batch-0 projects
                            xload(1, 0, T)
                for kt in range(NK):
                    v_sb = vvp.tile([128, HL * 65], BF16, name=f"V{b}_{kt}",
                                    tag=f"V{b}_{kt}")
                    pt = ps_main.tile([128, 128], BF16, name="p_tr", tag="ps")
                    nc.tensor.transpose(pt[:], vT[:, kt * 128:(kt + 1) * 128],
                                        id_sb[:])
                    v3 = v_sb[:].rearrange("p (h e) -> p h e", h=HL)
                    nc.gpsimd.tensor_copy(v3[:, :, 64], onesc[:])
                    nc.vector.tensor_copy(
                        v3[:, :, 0:64],
                        pt[:].rearrange("p (h e) -> p h e", h=HL),
                    )
                    V[b][kt] = v_sb
                    if kt % 4 == 3:
                        yield

            def attn_emit(h, b):
                """Attention for head-row h, batch b. Chunks are pairs of
                128-key blocks sharing one [128,1024] PSUM tile and one
                exp ACTIVATE; PV of chunk i issues after S/exp of chunk
                i+1 (one-deep software pipeline)."""
                h0 = h * 64
                # j's denominators at partition 32j so per-j DVE reciprocals
                # start on a legal partition boundary
                coll = wk.tile([100, 128], BF16, name="coll",
                               tag=f"coll{h}{b}", bufs=1)
                pending = None  # (p_sb, blocks, py, n_kt)

                def flush(pend):
                    p_sb, blocks, py, n_kt = pend
                    for ci, (kt, off) in enumerate(blocks):
                        base = 512 * ci
                        nc.tensor.matmul(
                            py[0:65, off:512],
                            V[b][kt][:, h * 65:(h + 1) * 65],
                            p_sb[:, base + off:base + 512],
                            start=(kt == 0),
                            stop=(kt == n_kt - 1),
                        )

                for j in range(NQ):
                    q0 = j * 512
                    py = ps_y.tile([65, 512], F32, name="p_y", tag="py")
                    n_kt = 4 * j + 4
                    # chunks: 1024-wide pairs of (kt, col_offset) blocks
                    chunks = []
                    for kt in range(0, 4 * j, 2):
                        chunks.append(((kt, 0), (kt + 1, 0)))
                    for i in (0, 2):
                        chunks.append(((4 * j + i, 128 * i),
                                       (4 * j + i + 1, 128 * (i + 1))))
                    for blocks in chunks:
                        pss = ps_ss.tile([128, 1024], F32, name="p_s",
                                         tag="pss")
                        lo = blocks[0][1]
                        for ci, (kt, off) in enumerate(blocks):
                            base = 512 * ci
                            # second block of a pair computes its full 512
                            # window so one ACTIVATE can span [lo, 1024)
                            # with no uninitialized PSUM gap; the extra
                            # above-diagonal columns are never read by PV
                            s_off = off if ci == 0 else 0
                            nc.tensor.matmul(
                                pss[:, base + s_off:base + 512],
                                kT[b][h0:h0 + 64, kt * 128:(kt + 1) * 128],
                                qT[b][h0:h0 + 64, q0 + s_off:q0 + 512],
                                start=True,
                                stop=True,
                            )
                        p_sb = wk.tile([128, 1024], BF16, name="p_sb",
                                       tag="p_sb", bufs=3)
                        nc.scalar.activation(
                            p_sb[:, lo:1024], pss[:, lo:1024], AF.Exp,
                            scale=float(SCALE),
                        )
                        for ci, (kt, off) in enumerate(blocks):
                            base = 512 * ci
                            if kt >= 4 * j:
                                nc.vector.tensor_mul(
                                    p_sb[:, base + off:base + off + 128],
                                    p_sb[:, base + off:base + off + 128],
                                    tri_sb[:],
                                )
                            if apply_pad_mask:
                                nc.vector.tensor_scalar_mul(
                                    p_sb[:, base + off:base + 512],
                                    p_sb[:, base + off:base + 512],
                                    padk_sb[:, b * NK + kt:b * NK + kt + 1],
                                )
                        if pending is not None:
                            flush(pending)
                            yield
                        pending = (p_sb, blocks, py, n_kt)
                    flush(pending)
                    pending = None
                    # evacuate PV accumulator: unnormalized y (rows 0:64)
                    # and the softmax denominator (row 64) in one cast
                    m = b * NQ + j
                    stage = wk.tile([65, 512], BF16, name="stage",
                                    tag="stage", bufs=4)
                    nc.vector.tensor_copy(stage[:], py[0:65, :])
                    nc.sync.dma_start(a2a_in[h][m, 0:64, :], stage[0:64, :])
                    nc.gpsimd.dma_start(coll[32 * j:32 * j + 4, :],
                                        stage[64:65, :])
                    yield
                # per-j reciprocal (not one wide op): rcol_j flushes to the
                # A2A payload as soon as its own denominators land, so the
                # last j's chain into the collective trigger is short
                rcol = wk.tile([100, 128], BF16, name="rcol",
                               tag=f"rcol{h}{b}", bufs=1)
                for j in range(NQ):
                    m = b * NQ + j
                    with nc.allow_low_precision(reason="bf16 softmax denom"):
                        nc.vector.reciprocal(rcol[32 * j:32 * j + 4, :],
                                             coll[32 * j:32 * j + 4, :])
                    nc.sync.dma_start(
                        a2a_in[h][m, 64, :].rearrange("(r c) -> r c", r=4),
                        rcol[32 * j:32 * j + 4, :],
                    )
                    yield

            wo_sb = xwp.tile([128, NCT * C], BF16, name="wo_sb")
            ytf = xwp.tile([128, N_CORES * 512], BF16, name="ytf")
            rsb = [None] * HL

            def wo_emit():
                # prefetch Wproj during attn(0,1)
                nc.sync.dma_start(
                    wo_sb[:].rearrange("p (c k) -> p c k", c=NCT),
                    wo[:].rearrange("(c p) k -> p c k", c=NCT),
                )
                yield

            def ytf_emit(h):
                # pull the h half of yT_full + recip rows once A2A h lands.
                # The tiny recip pulls go first and ride the scalar queue
                # (idle once its exps are done): they gate the pb broadcast
                # matmuls, while the big ytf pull only gates the later
                # vector multiplies — the two queues push in parallel.
                rsb[h] = xwp.tile([65, 4 * 512], BF16, name=f"rsb{h}",
                                  tag=f"rsb{h}")
                for g in (0, 1):
                    nc.scalar.dma_start(
                        rsb[h][64 * g:64 * g + 1, :].rearrange(
                            "p (s t) -> p s t", s=4),
                        a2a_out[h][4 * g:4 * g + 4, 64:65, :].rearrange(
                            "s p t -> p s t"),
                    )
                eng = nc.scalar if h == 0 else nc.sync
                eng.dma_start(
                    ytf[64 * h:64 * h + 64, :].rearrange(
                        "p (s t) -> p s t", s=N_CORES),
                    a2a_out[h][:, 0:64, :].rearrange("s p t -> p s t"),
                )
                yield

            def norm_emit(h):
                # normalize head-half h of yT_full in place: broadcast the
                # recip rows across its 64 partitions, one multiply per s.
                # h=0 runs during the second collective's window.
                ones = (ones_h0, ones_h1)[h]
                r0 = 64 * h
                for s in range(N_CORES):
                    base = 64 * (s // 4)
                    col = (s % 4) * 512
                    pb = ps_main.tile([128, 512], F32, name="p_b", tag="ps")
                    nc.tensor.matmul(pb[:], ones[base:base + 1, :],
                                     rsb[h][base:base + 1, col:col + 512],
                                     start=True, stop=True)
                    ys = ytf[r0:r0 + 64, s * 512:(s + 1) * 512]
                    nc.vector.tensor_mul(ys, ys, pb[r0:r0 + 64, :])
                    if s % 4 == 3:
                        yield

            def proj_emit():
                for mt in range(4):
                    o_sb = wk.tile([128, C], BF16, name="o_sb", tag="o_sb")
                    po2 = [ps_main.tile([128, 512], F32, name="p_o",
                                        tag="ps") for _ in range(2)]
                    for ct in range(NCT):
                        for n in range(2):
                            nc.tensor.matmul(
                                po2[n][:],
                                ytf[:, ct * 512 + mt * 128:
                                    ct * 512 + (mt + 1) * 128],
                                wo_sb[:, ct * C + n * 512:
                                      ct * C + (n + 1) * 512],
                                start=(ct == 0),
                                stop=(ct == NCT - 1),
                            )
                    for n in range(2):
                        nc.vector.tensor_copy(o_sb[:, n * 512:(n + 1) * 512],
                                              po2[n][:])
                        yield
                    nc.sync.dma_start(out[mt * 128:(mt + 1) * 128, :],
                                      o_sb[:])

            # ---------------- emission schedule ----------------
            _drain(qkv_emit(0))
            _drain(attn_emit(0, 0), qkv_emit(1))
            _drain(attn_emit(0, 1), wo_emit())
            nc.gpsimd.collective_compute(
                "AllToAll", mybir.AluOpType.bypass,
                replica_groups=[list(range(N_CORES))],
                ins=[a2a_in[0].opt().bitcast(F32)],
                outs=[a2a_out[0].opt().bitcast(F32)],
            )
            _drain(attn_emit(1, 0))
            _drain(attn_emit(1, 1))
            # The finale is pinned late via tile_wait_until: the scheduler
            # otherwise hoists these collective-gated instructions (ytf/rsb
            # pulls, pb broadcast matmuls) ahead of attn(1,*) work in the
            # in-order engine queues, stalling the PE ~50us on A2A#1.
            with tc.tile_wait_until(0.5):
                _drain(ytf_emit(0))
            nc.gpsimd.collective_compute(
                "AllToAll", mybir.AluOpType.bypass,
                replica_groups=[list(range(N_CORES))],
                ins=[a2a_in[1].opt().bitcast(F32)],
                outs=[a2a_out[1].opt().bitcast(F32)],
            )
            with tc.tile_wait_until(0.5):
                _drain(norm_emit(0))
            with tc.tile_wait_until(0.55):
                _drain(ytf_emit(1))
                _drain(norm_emit(1))
                _drain(proj_emit())

    nc.compile()
    return nc


def _host_inputs(x, tok_mask, Wqkv, Wproj, apply_pad_mask):
    bf = ml_dtypes.bfloat16
    x = np.ascontiguousarray(np.asarray(x, dtype=np.float32))
    Wqkv = np.ascontiguousarray(np.asarray(Wqkv, dtype=np.float32))
    Wproj = np.ascontiguousarray(np.asarray(Wproj, dtype=np.float32))
    xT = np.ascontiguousarray(
        np.concatenate([x[b].T for b in range(B)], axis=1)).astype(bf)
    wo_b = Wproj.astype(bf)
    r = np.arange(128)
    tri = (r[None, :] >= r[:, None]).astype(np.float32)  # keep if col >= row
    ident = np.eye(128, dtype=np.float32).astype(bf)
    if apply_pad_mask:
        padk = np.zeros((128, B * NK), np.float32)
        for b in range(B):
            padk[:, b * NK:(b + 1) * NK] = (
                np.asarray(tok_mask[b]).reshape(NK, 128).T.astype(np.float32)
            )
    else:
        padk = np.ones((128, B * NK), np.float32)

    in_maps = []
    for core in range(N_CORES):
        cols = slice(core * HL * D, (core + 1) * HL * D)
        wqkv_c = np.ascontiguousarray(
            np.concatenate(
                [Wqkv[:, :C][:, cols], Wqkv[:, C:2 * C][:, cols],
                 Wqkv[:, 2 * C:][:, cols]],
                axis=1,
            )
        ).astype(bf)
        in_maps.append(
            {
                "xT": xT,
                "wqkv": wqkv_c,
                "wo": wo_b,
                "tri": tri,
                "ident": ident,
                "padk": padk,
            }
        )
    return in_maps


def kernel(x, tok_mask, Wqkv, Wproj, _run_kwargs=None):
    tok = np.asarray(tok_mask)
    apply_pad_mask = not bool(tok.all())
    key = apply_pad_mask
    if key not in _BUILD_CACHE:
        _BUILD_CACHE[key] = build_kernel(apply_pad_mask)
    nc = _BUILD_CACHE[key]
    in_maps = _host_inputs(x, tok_mask, Wqkv, Wproj, apply_pad_mask)
    kw = dict(_run_kwargs or {})
    res = bass_utils.run_bass_kernel_spmd(
        nc, in_maps, core_ids=list(range(N_CORES)), **kw
    )
    out = np.empty((B, T, C), np.float32)
    for core in range(N_CORES):
        b, jj = divmod(core, NQ)
        out[b, jj * 512:(jj + 1) * 512, :] = np.asarray(
            res.results[core]["out"], dtype=np.float32)
    kernel.last_result = res
    return out
